# revision 3
# baseline (speedup 1.0000x reference)
"""TRN2 Bass kernel for nn_Att_mlp_sigmod (gnn message passing).

Reference computation:
    e = sigmoid(relu(h @ W1 + b1) @ W2 + b2)          # [N, 1]
    att[i, j] = e[j] * (graph_info[i, j] > 0)
    h_out = att @ h                                   # [N, D]
    returns (h_out, e)

Strategy (8 NeuronCores, SPMD, no collectives):
  - Row-shard graph_info: core c owns rows [c*1024, (c+1)*1024).
    h + MLP weights replicated; every core computes the full e locally.
  - Core output: houtT[d, i] = sum_j h[j, d] * attT[j, i], with
    attT[j, i] = (A[i, j] > 0) * e[j] built on the fly:
      A tile [i, j] -> bf16 mask (is_gt, exact 0/1) -> PE transpose (bf16,
      1 cyc/row) -> PSUM -> evacuate with per-partition scale by e[j]
      writing float32r (~1e-4 precision at full bf16 matmul speed).
  - Stationary operand: h rounded to float32r once.
  - Final unshard in numpy: transpose + concat of per-core houtT.
"""

import numpy as np
from contextlib import ExitStack

import concourse.bass as bass
import concourse.tile as tile
from concourse import bacc, mybir
from concourse.bass_utils import run_bass_kernel_spmd
from concourse.masks import make_identity
from concourse._compat import get_trn_type

N, D, HID = 8192, 256, 64
NCORES = 8
R = N // NCORES          # 1024 rows of A per core
JT = N // 128            # 64 j-tiles
NJC = N // 512           # 16 j-superchunks
NIC = R // 512           # 2 i-chunks per core
DB = D // 128            # 2 d-blocks

f32 = mybir.dt.float32
f32r = mybir.dt.float32r
bf16 = mybir.dt.bfloat16

AF = mybir.ActivationFunctionType
ALU = mybir.AluOpType

# dtype of the big matmul operands (attT moving + h stationary)
MAIN_DT = f32r
# dtype of the z = h @ W1 MLP matmul operands
Z_DT = f32r


def _emit(tc, a_d, h_d, w1_d, b1_d, w2_d, b2_d, houtT_d, e_d):
    nc = tc.nc
    ctx = ExitStack()

    sb = ctx.enter_context(tc.tile_pool(name="sb", bufs=1))
    ps = ctx.enter_context(tc.tile_pool(name="ps", bufs=1, space="PSUM"))

    # ---------------- constants ----------------
    ident_bf = sb.tile([128, 128], bf16, tag="ident_bf")
    make_identity(nc, ident_bf[:])
    ident_f = sb.tile([128, 128], f32, tag="ident_f")
    make_identity(nc, ident_f[:])

    w1_st = sb.tile([128, DB, HID], f32, tag="w1st")
    nc.sync.dma_start(w1_st[:], w1_d.rearrange("(b p) h -> p b h", p=128))
    w1_sb = sb.tile([128, DB, HID], Z_DT, tag="w1")
    nc.vector.tensor_copy(w1_sb[:], w1_st[:])
    b1_sb = sb.tile([HID, 1], f32, tag="b1")
    nc.sync.dma_start(b1_sb[:], b1_d)
    w2_sb = sb.tile([HID, 1], f32, tag="w2")
    nc.sync.dma_start(w2_sb[:], w2_d)
    b2_sb = sb.tile([1, 1], f32, tag="b2")
    nc.sync.dma_start(b2_sb[:], b2_d)

    # ------------- phase 0: stream h -> h_m (f32r) + MLP -> e -------------
    h_m = sb.tile([128, JT, D], MAIN_DT, tag="h_m")     # 64 KB/part stationary
    h_v = h_d.rearrange("(r p) d -> p r d", p=128)
    zT_sb = sb.tile([HID, N], f32, tag="zT")            # 32 KB/part

    for jc in range(NJC):
        hsts = []
        for s in range(4):
            hst = sb.tile([128, D], f32, tag="hst", bufs=10, name=f"hst_{jc}_{s}")
            nc.sync.dma_start(hst[:], h_v[:, 4 * jc + s, :])
            hsts.append(hst)
            # stationary copy rounded to f32r
            eng = nc.vector.tensor_copy if s % 2 == 0 else nc.scalar.copy
            eng(h_m[:, 4 * jc + s, :], hst[:])
        # z^T accumulation over the two 128-wide d blocks
        pz = ps.tile([HID, 512], f32, tag="pz", bufs=1, name=f"pz_{jc}")
        for b in range(DB):
            hTb = sb.tile([128, 512], Z_DT, tag="hTb", bufs=3, name=f"hTb_{jc}_{b}")
            for s in range(4):
                pt = ps.tile([128, 128], f32, tag="tp", bufs=3, name=f"tph_{jc}_{b}_{s}")
                nc.tensor.matmul(pt[:], hsts[s][:, 128 * b:128 * (b + 1)],
                                 ident_f[:], is_transpose=True)
                eng = nc.vector.tensor_copy if s % 2 == 0 else nc.scalar.copy
                eng(hTb[:, 128 * s:128 * (s + 1)], pt[:])
            nc.tensor.matmul(pz[:], w1_sb[:, b, :], hTb[:],
                             start=(b == 0), stop=(b == DB - 1))
        nc.scalar.activation(zT_sb[:, 512 * jc:512 * (jc + 1)], pz[:],
                             AF.Relu, bias=b1_sb[:], scale=1.0)

    # u column: u[j] = z^T[:, j] . W2  (64 matmuls, stationary = zT slice)
    pu = ps.tile([128, JT], f32, tag="pu", bufs=1)
    for r in range(JT):
        nc.tensor.matmul(pu[:, r:r + 1], zT_sb[:, 128 * r:128 * (r + 1)], w2_sb[:],
                         start=True, stop=True)

    # broadcast b2 across partitions via PE: ones[1,128].T @ b2[1,1]
    ones_sb = sb.tile([1, 128], f32, tag="ones")
    nc.vector.memset(ones_sb[:], 1.0)
    pbc = ps.tile([128, 1], f32, tag="pbc", bufs=1)
    nc.tensor.matmul(pbc[:], ones_sb[:], b2_sb[:], start=True, stop=True)
    b2b_sb = sb.tile([128, 1], f32, tag="b2b")
    nc.vector.tensor_copy(b2b_sb[:], pbc[:])

    e_sb = sb.tile([128, JT], f32, tag="e")
    nc.scalar.activation(e_sb[:], pu[:], AF.Sigmoid, bias=b2b_sb[:], scale=1.0)
    # e output: e_dram[r*128 + p] = e_sb[p, r]
    nc.sync.dma_start(e_d.rearrange("(r p) o -> p (r o)", p=128), e_sb[:])

    # ------------- phase 1: mask + transpose + scale + matmul -------------
    for ic in range(NIC):
        accs = [ps.tile([128, 512], f32, tag=f"acc{b}", bufs=1, name=f"acc{b}_{ic}")
                for b in range(DB)]
        for jt in range(NJC):
            ains = []
            for k in range(4):
                at = sb.tile([128, 512], f32, tag="a_in", bufs=8,
                             name=f"a_{ic}_{jt}_{k}")
                nc.sync.dma_start(
                    at[:],
                    a_d[512 * ic + 128 * k: 512 * ic + 128 * (k + 1),
                        512 * jt: 512 * (jt + 1)])
                ains.append(at)
            masks_ = []
            for k in range(4):
                mt = sb.tile([128, 512], bf16, tag="mask", bufs=8,
                             name=f"m_{ic}_{jt}_{k}")
                eng = nc.vector if k < 2 else nc.gpsimd
                eng.tensor_scalar(mt[:], ains[k][:], 0.0, None, ALU.is_gt)
                masks_.append(mt)
            for q in range(4):
                j = 4 * jt + q
                aT = sb.tile([128, 512], MAIN_DT, tag="attT", bufs=6,
                             name=f"attT_{ic}_{j}")
                for k in range(4):
                    pt = ps.tile([128, 128], bf16, tag="tp", bufs=3,
                                 name=f"tp_{ic}_{j}_{k}")
                    nc.tensor.matmul(pt[:], masks_[k][:, 128 * q:128 * (q + 1)],
                                     ident_bf[:], is_transpose=True)
                    # evacuate with per-partition (j) scale by e[j]
                    if k % 2 == 0:
                        nc.vector.tensor_scalar_mul(
                            aT[:, 128 * k:128 * (k + 1)], pt[:], e_sb[:, j:j + 1])
                    else:
                        nc.scalar.mul(
                            aT[:, 128 * k:128 * (k + 1)], pt[:], e_sb[:, j:j + 1])
                for b in range(DB):
                    nc.tensor.matmul(accs[b][:],
                                     h_m[:, j, 128 * b:128 * (b + 1)], aT[:],
                                     start=(j == 0), stop=(j == JT - 1))
        for b in range(DB):
            ot = sb.tile([128, 512], f32, tag="hout", bufs=4,
                         name=f"ot_{ic}_{b}")
            nc.vector.tensor_copy(ot[:], accs[b][:])
            nc.sync.dma_start(
                houtT_d[128 * b:128 * (b + 1), 512 * ic:512 * (ic + 1)], ot[:])

    ctx.close()


_built = None


def _build():
    global _built
    if _built is not None:
        return _built
    nc = bacc.Bacc(get_trn_type(), target_bir_lowering=False, debug=False)
    a_d = nc.dram_tensor("a_shard", [R, N], f32, kind="ExternalInput").ap()
    h_d = nc.dram_tensor("h_full", [N, D], f32, kind="ExternalInput").ap()
    w1_d = nc.dram_tensor("w1", [D, HID], f32, kind="ExternalInput").ap()
    b1_d = nc.dram_tensor("b1", [HID, 1], f32, kind="ExternalInput").ap()
    w2_d = nc.dram_tensor("w2", [HID, 1], f32, kind="ExternalInput").ap()
    b2_d = nc.dram_tensor("b2", [1, 1], f32, kind="ExternalInput").ap()
    houtT_d = nc.dram_tensor("houtT", [D, R], f32, kind="ExternalOutput").ap()
    e_d = nc.dram_tensor("e_out", [N, 1], f32, kind="ExternalOutput").ap()

    with tile.TileContext(nc) as tc:
        _emit(tc, a_d, h_d, w1_d, b1_d, w2_d, b2_d, houtT_d, e_d)
    nc.compile()
    _built = nc
    return nc


def _run(nc, graph_info, h, W1, b1, W2, b2, **run_kwargs):
    in_maps = []
    for c in range(NCORES):
        in_maps.append({
            "a_shard": np.ascontiguousarray(graph_info[c * R:(c + 1) * R]),
            "h_full": h,
            "w1": W1,
            "b1": b1.reshape(HID, 1),
            "w2": W2.reshape(HID, 1),
            "b2": b2.reshape(1, 1),
        })
    res = run_bass_kernel_spmd(nc, in_maps, list(range(NCORES)), **run_kwargs)
    return res


def kernel(graph_info, h, W1, b1, W2, b2):
    graph_info = np.asarray(graph_info, dtype=np.float32)
    h = np.asarray(h, dtype=np.float32)
    W1 = np.asarray(W1, dtype=np.float32)
    b1 = np.asarray(b1, dtype=np.float32)
    W2 = np.asarray(W2, dtype=np.float32)
    b2 = np.asarray(b2, dtype=np.float32)

    nc = _build()
    res = _run(nc, graph_info, h, W1, b1, W2, b2)
    h_out = np.concatenate([r["houtT"].T for r in res.results], axis=0)
    e = res.results[0]["e_out"]
    return (h_out, e)


# revision 8
# speedup vs baseline: 2.3045x; 2.3045x over previous
"""TRN2 Bass kernel for nn_Att_mlp_sigmod (gnn message passing).

Reference computation:
    e = sigmoid(relu(h @ W1 + b1) @ W2 + b2)          # [N, 1]
    att[i, j] = e[j] * (graph_info[i, j] > 0)
    h_out = att @ h                                   # [N, D]
    returns (h_out, e)

Strategy (8 NeuronCores, SPMD, no collectives):
  - Row-shard graph_info: core c owns rows [c*1024, (c+1)*1024).
    h + MLP weights replicated; every core computes the full e locally.
  - Core output: houtT[d, i] = sum_j g[j, d] * maskT[j, i] where
    g = e * h (float32r, ~1e-4 precision at full bf16 matmul speed) and
    maskT = (A^T > 0) in exact {0,1}.
  - maskT tiles are built by PE-transposing raw f32 A tiles (4 transposes
    fill one PSUM bank) and evacuating each bank with a single fused
    compare op: DVE tensor_scalar(is_gt) or ACT Sign (split across both
    engines, one [128,512] op per bank — per-op overhead amortized).
  - h is loaded once, rounded to f32r (h_m), transposed on PE for the MLP,
    then scaled by e in place (g) for the main matmul stationary.
  - Final unshard in numpy: transpose + concat of per-core houtT.
"""

import hashlib
import os
import shutil
import numpy as np
from contextlib import ExitStack

import concourse.bass as bass
import concourse.tile as tile
from concourse import bacc, mybir
from concourse.bass_utils import run_bass_kernel_spmd
from concourse.masks import make_identity
from concourse._compat import get_trn_type

# Cache compiled NEFFs by BIR hash — walrus compile of this kernel takes
# minutes and bass2jax recompiles in a fresh tempdir on every process start.
import concourse.bass2jax as _b2j

_NEFF_CACHE = "/tmp/bass_neff_cache"

if not getattr(_b2j, "_ant_neff_cache_installed", False):
    _orig_cbk = _b2j.compile_bir_kernel

    def _cached_cbk(bir_json, tmpdir, neff_name="file.neff"):
        os.makedirs(_NEFF_CACHE, exist_ok=True)
        key = hashlib.sha256(bir_json).hexdigest()[:24]
        cpath = os.path.join(_NEFF_CACHE, f"{key}.neff")
        dst = os.path.join(tmpdir, neff_name)
        if os.path.exists(cpath):
            shutil.copy(cpath, dst)
            return dst
        neff = _orig_cbk(bir_json, tmpdir, neff_name)
        tmp = f"{cpath}.tmp{os.getpid()}"
        shutil.copy(neff, tmp)
        os.replace(tmp, cpath)
        return neff

    _b2j.compile_bir_kernel = _cached_cbk
    _b2j._ant_neff_cache_installed = True

N, D, HID = 8192, 256, 64
NCORES = 8
R = N // NCORES          # 1024 rows of A per core
JT = N // 128            # 64 j-tiles
NJC = N // 512           # 16 j-superchunks
NIC = R // 512           # 2 i-chunks per core
DB = D // 128            # 2 d-blocks

f32 = mybir.dt.float32
f32r = mybir.dt.float32r
bf16 = mybir.dt.bfloat16

AF = mybir.ActivationFunctionType
ALU = mybir.AluOpType

# fraction of A-bank evacuations on ACT (Sign); rest on DVE (is_gt)
ACT_EVAC_MOD = 2          # k % 2 == 1 -> ACT


def _emit(tc, a_d, h_d, w1_d, b1_d, w2_d, b2_d, houtT_d, e_d):
    nc = tc.nc
    ctx = ExitStack()

    sb = ctx.enter_context(tc.tile_pool(name="sb", bufs=1))
    ps = ctx.enter_context(tc.tile_pool(name="ps", bufs=1, space="PSUM"))

    # ---------------- constants ----------------
    ident_f = sb.tile([128, 128], f32, tag="ident_f")
    make_identity(nc, ident_f[:])
    ident_r = sb.tile([128, 128], f32r, tag="ident_r")
    nc.vector.tensor_copy(ident_r[:], ident_f[:])

    w1_st = sb.tile([128, DB, HID], f32, tag="w1st")
    nc.sync.dma_start(w1_st[:], w1_d.rearrange("(b p) h -> p b h", p=128))
    w1_sb = sb.tile([128, DB, HID], f32r, tag="w1")
    nc.vector.tensor_copy(w1_sb[:], w1_st[:])
    b1_sb = sb.tile([HID, 1], f32, tag="b1")
    nc.sync.dma_start(b1_sb[:], b1_d)
    w2_sb = sb.tile([HID, 1], f32, tag="w2")
    nc.sync.dma_start(w2_sb[:], w2_d)
    b2_sb = sb.tile([1, 1], f32, tag="b2")
    nc.sync.dma_start(b2_sb[:], b2_d)

    # ------------- phase 0: h -> h_m (f32r); MLP -> e; g = e*h_m -------------
    h_m = sb.tile([128, JT, D], f32r, tag="h_m")        # 64 KB/part
    h_v = h_d.rearrange("(r p) d -> p r d", p=128)
    for t in range(16):
        hst = sb.tile([128, 4, D], f32, tag="hst", bufs=6, name=f"hst_{t}")
        nc.sync.dma_start(hst[:], h_v[:, 4 * t:4 * (t + 1), :])
        # round to f32r (big op: [128, 1024])
        nc.vector.tensor_copy(h_m[:, 4 * t:4 * (t + 1), :], hst[:])

    zT_sb = sb.tile([HID, N], f32, tag="zT")            # 32 KB/part
    for jc in range(NJC):
        pz = ps.tile([HID, 512], f32, tag="pz", bufs=1, name=f"pz_{jc}")
        for b in range(DB):
            # hT chunk [128 d, 512 j] from 4 PE transposes into one bank
            pt = ps.tile([128, 512], f32r, tag="tp", bufs=3, name=f"tph_{jc}_{b}")
            for s in range(4):
                nc.tensor.matmul(pt[:, 128 * s:128 * (s + 1)],
                                 h_m[:, 4 * jc + s, 128 * b:128 * (b + 1)],
                                 ident_r[:], is_transpose=True)
            hTb = sb.tile([128, 512], f32r, tag="hTb", bufs=3, name=f"hTb_{jc}_{b}")
            nc.scalar.copy(hTb[:], pt[:])
            nc.tensor.matmul(pz[:], w1_sb[:, b, :], hTb[:],
                             start=(b == 0), stop=(b == DB - 1))
        # z^T = relu(pz + b1), rounded to f32r (DVE fused add+max)
        nc.vector.tensor_scalar(zT_sb[:, 512 * jc:512 * (jc + 1)], pz[:],
                                b1_sb[:], 0.0, ALU.add, ALU.max)

    # u column: u[j] = z^T[:, j] . W2  (64 matmuls, stationary = zT slice)
    pu = ps.tile([128, JT], f32, tag="pu", bufs=1)
    for r in range(JT):
        nc.tensor.matmul(pu[:, r:r + 1], zT_sb[:, 128 * r:128 * (r + 1)], w2_sb[:],
                         start=True, stop=True)

    # broadcast b2 across partitions via PE: ones[1,128].T @ b2[1,1]
    ones_sb = sb.tile([1, 128], f32, tag="ones")
    nc.vector.memset(ones_sb[:], 1.0)
    pbc = ps.tile([128, 1], f32, tag="pbc", bufs=1)
    nc.tensor.matmul(pbc[:], ones_sb[:], b2_sb[:], start=True, stop=True)
    b2b_sb = sb.tile([128, 1], f32, tag="b2b")
    nc.vector.tensor_copy(b2b_sb[:], pbc[:])

    e_sb = sb.tile([128, JT], f32, tag="e")
    nc.scalar.activation(e_sb[:], pu[:], AF.Sigmoid, bias=b2b_sb[:], scale=1.0)
    # e output: e_dram[r*128 + p] = e_sb[p, r]
    nc.sync.dma_start(e_d.rearrange("(r p) o -> p (r o)", p=128), e_sb[:])

    # g = e * h_m in place (per-partition scale, 4 big ops)
    for t in range(4):
        nc.vector.tensor_tensor(
            h_m[:, 16 * t:16 * (t + 1), :],
            h_m[:, 16 * t:16 * (t + 1), :],
            e_sb[:, 16 * t:16 * (t + 1), None].to_broadcast([128, 16, D]),
            ALU.mult)

    # ------------- phase 1: transpose A + fused mask evac + matmul -------------
    for ic in range(NIC):
        accs = [ps.tile([128, 512], f32, tag=f"acc{b}", bufs=1, name=f"acc{b}_{ic}")
                for b in range(DB)]
        for jt in range(NJC):
            ains = []
            for k in range(4):
                at = sb.tile([128, 512], f32, tag="a_in", bufs=8,
                             name=f"a_{ic}_{jt}_{k}")
                nc.sync.dma_start(
                    at[:],
                    a_d[512 * ic + 128 * k: 512 * ic + 128 * (k + 1),
                        512 * jt: 512 * (jt + 1)])
                ains.append(at)
            for q in range(4):
                j = 4 * jt + q
                pt = ps.tile([128, 512], f32, tag="tp", bufs=3,
                             name=f"tp_{ic}_{j}")
                for k in range(4):
                    nc.tensor.matmul(pt[:, 128 * k:128 * (k + 1)],
                                     ains[k][:, 128 * q:128 * (q + 1)],
                                     ident_f[:], is_transpose=True)
                aT = sb.tile([128, 512], f32r, tag="attT", bufs=6,
                             name=f"attT_{ic}_{j}")
                if q % ACT_EVAC_MOD == 1:
                    nc.scalar.sign(aT[:], pt[:])
                else:
                    nc.vector.tensor_scalar(aT[:], pt[:], 0.0, None, ALU.is_gt)
                for b in range(DB):
                    nc.tensor.matmul(accs[b][:],
                                     h_m[:, j, 128 * b:128 * (b + 1)], aT[:],
                                     start=(j == 0), stop=(j == JT - 1))
        for b in range(DB):
            ot = sb.tile([128, 512], f32, tag="hout", bufs=4,
                         name=f"ot_{ic}_{b}")
            nc.vector.tensor_copy(ot[:], accs[b][:])
            nc.sync.dma_start(
                houtT_d[128 * b:128 * (b + 1), 512 * ic:512 * (ic + 1)], ot[:])

    ctx.close()


_built = None


def _build():
    global _built
    if _built is not None:
        return _built
    nc = bacc.Bacc(get_trn_type(), target_bir_lowering=False, debug=False)
    a_d = nc.dram_tensor("a_shard", [R, N], f32, kind="ExternalInput").ap()
    h_d = nc.dram_tensor("h_full", [N, D], f32, kind="ExternalInput").ap()
    w1_d = nc.dram_tensor("w1", [D, HID], f32, kind="ExternalInput").ap()
    b1_d = nc.dram_tensor("b1", [HID, 1], f32, kind="ExternalInput").ap()
    w2_d = nc.dram_tensor("w2", [HID, 1], f32, kind="ExternalInput").ap()
    b2_d = nc.dram_tensor("b2", [1, 1], f32, kind="ExternalInput").ap()
    houtT_d = nc.dram_tensor("houtT", [D, R], f32, kind="ExternalOutput").ap()
    e_d = nc.dram_tensor("e_out", [N, 1], f32, kind="ExternalOutput").ap()

    with tile.TileContext(nc) as tc:
        _emit(tc, a_d, h_d, w1_d, b1_d, w2_d, b2_d, houtT_d, e_d)
    nc.compile()
    _built = nc
    return nc


def _run(nc, graph_info, h, W1, b1, W2, b2, **run_kwargs):
    in_maps = []
    for c in range(NCORES):
        in_maps.append({
            "a_shard": np.ascontiguousarray(graph_info[c * R:(c + 1) * R]),
            "h_full": h,
            "w1": W1,
            "b1": b1.reshape(HID, 1),
            "w2": W2.reshape(HID, 1),
            "b2": b2.reshape(1, 1),
        })
    res = run_bass_kernel_spmd(nc, in_maps, list(range(NCORES)), **run_kwargs)
    return res


def kernel(graph_info, h, W1, b1, W2, b2):
    graph_info = np.asarray(graph_info, dtype=np.float32)
    h = np.asarray(h, dtype=np.float32)
    W1 = np.asarray(W1, dtype=np.float32)
    b1 = np.asarray(b1, dtype=np.float32)
    W2 = np.asarray(W2, dtype=np.float32)
    b2 = np.asarray(b2, dtype=np.float32)

    nc = _build()
    res = _run(nc, graph_info, h, W1, b1, W2, b2)
    h_out = np.concatenate([r["houtT"].T for r in res.results], axis=0)
    e = res.results[0]["e_out"]
    return (h_out, e)


# revision 11
# speedup vs baseline: 2.4369x; 1.0574x over previous
"""TRN2 Bass kernel for nn_Att_mlp_sigmod (gnn message passing).

Reference computation:
    e = sigmoid(relu(h @ W1 + b1) @ W2 + b2)          # [N, 1]
    att[i, j] = e[j] * (graph_info[i, j] > 0)
    h_out = att @ h                                   # [N, D]
    returns (h_out, e)

Strategy (8 NeuronCores, SPMD, no collectives):
  - Row-shard graph_info: core c owns rows [c*1024, (c+1)*1024).
    h + MLP weights replicated; every core computes the full e locally.
  - Core output: houtT[d, i] = sum_j g[j, d] * maskT[j, i] where
    g = e * h (float32r, ~1e-4 precision at full bf16 matmul speed) and
    maskT = (A^T > 0) in exact {0,1}.
  - A tiles are first compared to 0 into exact {0,1} bf16 masks (big DVE
    is_gt / ACT Sign ops), then transposed on the PE as REGULAR matmuls
    against a bf16 identity (1 cyc/row + FWL, and regular matmuls keep the
    PE HAM clock warm — is_transpose ops count as idle and locked V3 at
    1.2 GHz). The f32 PSUM result is exact 0/1; each [128,512] PSUM bank
    is evacuated with a single per-partition multiply by e[j] (DVE
    tensor_scalar_mul / ACT activation-scale), writing f32r attT.
  - h is loaded once, rounded to f32r (h_m): the main matmul stationary.
  - Final unshard in numpy: transpose + concat of per-core houtT.
"""

import hashlib
import os
import shutil
import numpy as np
from contextlib import ExitStack

import concourse.bass as bass
import concourse.tile as tile
from concourse import bacc, mybir
from concourse.bass_utils import run_bass_kernel_spmd
from concourse.masks import make_identity
from concourse._compat import get_trn_type

# Cache compiled NEFFs by BIR hash — walrus compile of this kernel takes
# minutes and bass2jax recompiles in a fresh tempdir on every process start.
import concourse.bass2jax as _b2j

_NEFF_CACHE = "/tmp/bass_neff_cache"

if not getattr(_b2j, "_ant_neff_cache_installed", False):
    _orig_cbk = _b2j.compile_bir_kernel

    def _cached_cbk(bir_json, tmpdir, neff_name="file.neff"):
        os.makedirs(_NEFF_CACHE, exist_ok=True)
        key = hashlib.sha256(bir_json).hexdigest()[:24]
        cpath = os.path.join(_NEFF_CACHE, f"{key}.neff")
        dst = os.path.join(tmpdir, neff_name)
        if os.path.exists(cpath):
            shutil.copy(cpath, dst)
            return dst
        neff = _orig_cbk(bir_json, tmpdir, neff_name)
        tmp = f"{cpath}.tmp{os.getpid()}"
        shutil.copy(neff, tmp)
        os.replace(tmp, cpath)
        return neff

    _b2j.compile_bir_kernel = _cached_cbk
    _b2j._ant_neff_cache_installed = True

N, D, HID = 8192, 256, 64
NCORES = 8
R = N // NCORES          # 1024 rows of A per core
JT = N // 128            # 64 j-tiles
NJC = N // 512           # 16 j-superchunks
NIC = R // 512           # 2 i-chunks per core
DB = D // 128            # 2 d-blocks

f32 = mybir.dt.float32
f32r = mybir.dt.float32r
bf16 = mybir.dt.bfloat16

AF = mybir.ActivationFunctionType
ALU = mybir.AluOpType

# fraction of A-bank evacuations on ACT (Sign); rest on DVE (is_gt)
ACT_EVAC_MOD = 2          # k % 2 == 1 -> ACT


def _emit(tc, a_d, h_d, w1_d, b1_d, w2_d, b2_d, houtT_d, e_d):
    nc = tc.nc
    ctx = ExitStack()

    sb = ctx.enter_context(tc.tile_pool(name="sb", bufs=1))
    ps = ctx.enter_context(tc.tile_pool(name="ps", bufs=1, space="PSUM"))

    # ---------------- constants ----------------
    ident_f = sb.tile([128, 128], f32, tag="ident_f")
    make_identity(nc, ident_f[:])
    ident_r = sb.tile([128, 128], f32r, tag="ident_r")
    nc.vector.tensor_copy(ident_r[:], ident_f[:])
    ident_bf = sb.tile([128, 128], bf16, tag="ident_bf")
    nc.vector.tensor_copy(ident_bf[:], ident_f[:])

    w1_st = sb.tile([128, DB, HID], f32, tag="w1st")
    nc.sync.dma_start(w1_st[:], w1_d.rearrange("(b p) h -> p b h", p=128))
    w1_sb = sb.tile([128, DB, HID], f32r, tag="w1")
    nc.vector.tensor_copy(w1_sb[:], w1_st[:])
    b1_sb = sb.tile([HID, 1], f32, tag="b1")
    nc.sync.dma_start(b1_sb[:], b1_d)
    w2_sb = sb.tile([HID, 1], f32, tag="w2")
    nc.sync.dma_start(w2_sb[:], w2_d)
    b2_sb = sb.tile([1, 1], f32, tag="b2")
    nc.sync.dma_start(b2_sb[:], b2_d)

    # ------------- phase 0: h -> h_m (f32r); MLP -> e; g = e*h_m -------------
    h_m = sb.tile([128, JT, D], f32r, tag="h_m")        # 64 KB/part
    h_v = h_d.rearrange("(r p) d -> p r d", p=128)
    for t in range(16):
        hst = sb.tile([128, 4, D], f32, tag="hst", bufs=6, name=f"hst_{t}")
        nc.sync.dma_start(hst[:], h_v[:, 4 * t:4 * (t + 1), :])
        # round to f32r (big op: [128, 1024])
        nc.vector.tensor_copy(h_m[:, 4 * t:4 * (t + 1), :], hst[:])

    zT_sb = sb.tile([HID, N], f32, tag="zT")            # 32 KB/part
    for jc in range(NJC):
        pz = ps.tile([HID, 512], f32, tag="pz", bufs=1, name=f"pz_{jc}")
        for b in range(DB):
            # hT chunk [128 d, 512 j] from 4 PE transposes into one bank
            pt = ps.tile([128, 512], f32r, tag="tp", bufs=3, name=f"tph_{jc}_{b}")
            for s in range(4):
                nc.tensor.matmul(pt[:, 128 * s:128 * (s + 1)],
                                 h_m[:, 4 * jc + s, 128 * b:128 * (b + 1)],
                                 ident_r[:], is_transpose=True)
            hTb = sb.tile([128, 512], f32r, tag="hTb", bufs=3, name=f"hTb_{jc}_{b}")
            nc.scalar.copy(hTb[:], pt[:])
            nc.tensor.matmul(pz[:], w1_sb[:, b, :], hTb[:],
                             start=(b == 0), stop=(b == DB - 1))
        # z^T = relu(pz + b1), rounded to f32r (DVE fused add+max)
        nc.vector.tensor_scalar(zT_sb[:, 512 * jc:512 * (jc + 1)], pz[:],
                                b1_sb[:], 0.0, ALU.add, ALU.max)

    # u column: u[j] = z^T[:, j] . W2  (64 matmuls, stationary = zT slice)
    pu = ps.tile([128, JT], f32, tag="pu", bufs=1)
    for r in range(JT):
        nc.tensor.matmul(pu[:, r:r + 1], zT_sb[:, 128 * r:128 * (r + 1)], w2_sb[:],
                         start=True, stop=True)

    # broadcast b2 across partitions via PE: ones[1,128].T @ b2[1,1]
    ones_sb = sb.tile([1, 128], f32, tag="ones")
    nc.vector.memset(ones_sb[:], 1.0)
    pbc = ps.tile([128, 1], f32, tag="pbc", bufs=1)
    nc.tensor.matmul(pbc[:], ones_sb[:], b2_sb[:], start=True, stop=True)
    b2b_sb = sb.tile([128, 1], f32, tag="b2b")
    nc.vector.tensor_copy(b2b_sb[:], pbc[:])

    e_sb = sb.tile([128, JT], f32, tag="e")
    nc.scalar.activation(e_sb[:], pu[:], AF.Sigmoid, bias=b2b_sb[:], scale=1.0)
    # e output: e_dram[r*128 + p] = e_sb[p, r]
    nc.sync.dma_start(e_d.rearrange("(r p) o -> p (r o)", p=128), e_sb[:])

    # ------- phase 1: bf16 mask, regular-mm transpose, e-scaled evac, matmul -------
    for ic in range(NIC):
        accs = [ps.tile([128, 512], f32, tag=f"acc{b}", bufs=1, name=f"acc{b}_{ic}")
                for b in range(DB)]
        for jt in range(NJC):
            masks_ = []
            for k in range(4):
                at = sb.tile([128, 512], f32, tag="a_in", bufs=8,
                             name=f"a_{ic}_{jt}_{k}")
                nc.sync.dma_start(
                    at[:],
                    a_d[512 * ic + 128 * k: 512 * ic + 128 * (k + 1),
                        512 * jt: 512 * (jt + 1)])
                mt = sb.tile([128, 512], bf16, tag="mask", bufs=8,
                             name=f"m_{ic}_{jt}_{k}")
                if (jt * 4 + k) % 8 < 3:
                    nc.vector.tensor_scalar(mt[:], at[:], 0.0, None, ALU.is_gt)
                else:
                    nc.scalar.sign(mt[:], at[:])
                masks_.append(mt)
            for q in range(4):
                j = 4 * jt + q
                pt = ps.tile([128, 512], f32, tag="tp", bufs=3,
                             name=f"tp_{ic}_{j}")
                for k in range(4):
                    # regular matmul transpose: out = mask.T @ I (keeps HAM warm)
                    nc.tensor.matmul(pt[:, 128 * k:128 * (k + 1)],
                                     masks_[k][:, 128 * q:128 * (q + 1)],
                                     ident_bf[:], start=True, stop=True)
                aT = sb.tile([128, 512], f32r, tag="attT", bufs=6,
                             name=f"attT_{ic}_{j}")
                # evacuate with per-partition scale by e[j] (mask is exact 0/1)
                if q % 2 == 1:
                    nc.scalar.mul(aT[:], pt[:], e_sb[:, j:j + 1])
                else:
                    nc.vector.tensor_scalar_mul(aT[:], pt[:], e_sb[:, j:j + 1])
                for b in range(DB):
                    nc.tensor.matmul(accs[b][:],
                                     h_m[:, j, 128 * b:128 * (b + 1)], aT[:],
                                     start=(j == 0), stop=(j == JT - 1))
        for b in range(DB):
            ot = sb.tile([128, 512], f32, tag="hout", bufs=4,
                         name=f"ot_{ic}_{b}")
            nc.vector.tensor_copy(ot[:], accs[b][:])
            nc.sync.dma_start(
                houtT_d[128 * b:128 * (b + 1), 512 * ic:512 * (ic + 1)], ot[:])

    ctx.close()


_built = None


def _build():
    global _built
    if _built is not None:
        return _built
    nc = bacc.Bacc(get_trn_type(), target_bir_lowering=False, debug=False)
    a_d = nc.dram_tensor("a_shard", [R, N], f32, kind="ExternalInput").ap()
    h_d = nc.dram_tensor("h_full", [N, D], f32, kind="ExternalInput").ap()
    w1_d = nc.dram_tensor("w1", [D, HID], f32, kind="ExternalInput").ap()
    b1_d = nc.dram_tensor("b1", [HID, 1], f32, kind="ExternalInput").ap()
    w2_d = nc.dram_tensor("w2", [HID, 1], f32, kind="ExternalInput").ap()
    b2_d = nc.dram_tensor("b2", [1, 1], f32, kind="ExternalInput").ap()
    houtT_d = nc.dram_tensor("houtT", [D, R], f32, kind="ExternalOutput").ap()
    e_d = nc.dram_tensor("e_out", [N, 1], f32, kind="ExternalOutput").ap()

    with tile.TileContext(nc) as tc:
        _emit(tc, a_d, h_d, w1_d, b1_d, w2_d, b2_d, houtT_d, e_d)
    nc.compile()
    _built = nc
    return nc


def _run(nc, graph_info, h, W1, b1, W2, b2, **run_kwargs):
    in_maps = []
    for c in range(NCORES):
        in_maps.append({
            "a_shard": np.ascontiguousarray(graph_info[c * R:(c + 1) * R]),
            "h_full": h,
            "w1": W1,
            "b1": b1.reshape(HID, 1),
            "w2": W2.reshape(HID, 1),
            "b2": b2.reshape(1, 1),
        })
    res = run_bass_kernel_spmd(nc, in_maps, list(range(NCORES)), **run_kwargs)
    return res


def kernel(graph_info, h, W1, b1, W2, b2):
    graph_info = np.asarray(graph_info, dtype=np.float32)
    h = np.asarray(h, dtype=np.float32)
    W1 = np.asarray(W1, dtype=np.float32)
    b1 = np.asarray(b1, dtype=np.float32)
    W2 = np.asarray(W2, dtype=np.float32)
    b2 = np.asarray(b2, dtype=np.float32)

    nc = _build()
    res = _run(nc, graph_info, h, W1, b1, W2, b2)
    h_out = np.concatenate([r["houtT"].T for r in res.results], axis=0)
    e = res.results[0]["e_out"]
    return (h_out, e)


# revision 14
# speedup vs baseline: 2.4370x; 1.0000x over previous
"""TRN2 Bass kernel for nn_Att_mlp_sigmod (gnn message passing).

Reference computation:
    e = sigmoid(relu(h @ W1 + b1) @ W2 + b2)          # [N, 1]
    att[i, j] = e[j] * (graph_info[i, j] > 0)
    h_out = att @ h                                   # [N, D]
    returns (h_out, e)

Strategy (8 NeuronCores, SPMD):
  - Row-shard graph_info (core c owns rows [c*1024, (c+1)*1024)); replicate
    h + weights. The tiny MLP is sharded by node: each core computes e for
    its own 1024 nodes, then one AllGather of [1024] floats assembles the
    full e everywhere.
  - Core output: houtT[d, i] = sum_j h_m[j, d] * attT[j, i] with h_m = h
    rounded to float32r (stationary; f32r runs at bf16 speed for moving
    free >= 256 and gives ~1e-4 precision) and attT = (A^T > 0) * e[j].
  - attT build: A tiles -> exact {0,1} bf16 masks (big DVE is_gt / ACT
    Sign ops) -> transposed on the PE as REGULAR matmuls against a bf16
    identity (1 cyc/row + FWL; regular matmuls keep the HAM clock at 2.4
    GHz, while is_transpose-mode ops count as idle and halve the clock) ->
    each [128,512] f32 PSUM bank evacuated by ONE per-partition multiply
    with e[j] (DVE tensor_scalar_mul / ACT activation-scale) into f32r.
  - Final unshard in numpy: transpose + concat of per-core houtT.
"""

import hashlib
import os
import shutil
import numpy as np
from contextlib import ExitStack

import concourse.bass as bass
import concourse.tile as tile
from concourse import bacc, mybir
from concourse.bass_utils import run_bass_kernel_spmd
from concourse.masks import make_identity
from concourse._compat import get_trn_type

# Cache compiled NEFFs by BIR hash — walrus compile of this kernel takes
# minutes and bass2jax recompiles in a fresh tempdir on every process start.
import concourse.bass2jax as _b2j

_NEFF_CACHE = "/tmp/bass_neff_cache"

if not getattr(_b2j, "_ant_neff_cache_installed", False):
    _orig_cbk = _b2j.compile_bir_kernel

    def _cached_cbk(bir_json, tmpdir, neff_name="file.neff"):
        os.makedirs(_NEFF_CACHE, exist_ok=True)
        key = hashlib.sha256(bir_json).hexdigest()[:24]
        cpath = os.path.join(_NEFF_CACHE, f"{key}.neff")
        dst = os.path.join(tmpdir, neff_name)
        if os.path.exists(cpath):
            shutil.copy(cpath, dst)
            return dst
        neff = _orig_cbk(bir_json, tmpdir, neff_name)
        tmp = f"{cpath}.tmp{os.getpid()}"
        shutil.copy(neff, tmp)
        os.replace(tmp, cpath)
        return neff

    _b2j.compile_bir_kernel = _cached_cbk
    _b2j._ant_neff_cache_installed = True

N, D, HID = 8192, 256, 64
NCORES = 8
R = N // NCORES          # 1024 rows of A per core
JT = N // 128            # 64 j-tiles
NJC2 = N // 1024         # 8 j-chunks of 1024
NIC = R // 512           # 2 i-chunks per core
DB = D // 128            # 2 d-blocks
LJT = R // 128           # 8 local j-tiles for the sharded MLP

f32 = mybir.dt.float32
f32r = mybir.dt.float32r
bf16 = mybir.dt.bfloat16

AF = mybir.ActivationFunctionType
ALU = mybir.AluOpType


def _emit(tc, a_d, h_d, hm_d, w1_d, b1_d, w2_d, b2_d, houtT_d, e_d):
    nc = tc.nc
    ctx = ExitStack()

    sb = ctx.enter_context(tc.tile_pool(name="sb", bufs=1))
    ps = ctx.enter_context(tc.tile_pool(name="ps", bufs=1, space="PSUM"))
    dram = ctx.enter_context(tc.tile_pool(name="dram", bufs=1, space="DRAM"))

    # ---------------- constants ----------------
    ident_f = sb.tile([128, 128], f32, tag="ident_f")
    make_identity(nc, ident_f[:])
    ident_r = sb.tile([128, 128], f32r, tag="ident_r")
    nc.vector.tensor_copy(ident_r[:], ident_f[:])
    ident_bf = sb.tile([128, 128], bf16, tag="ident_bf")
    nc.vector.tensor_copy(ident_bf[:], ident_f[:])

    w1_st = sb.tile([128, DB, HID], f32, tag="w1st")
    nc.sync.dma_start(w1_st[:], w1_d.rearrange("(b p) h -> p b h", p=128))
    w1_sb = sb.tile([128, DB, HID], f32r, tag="w1")
    nc.vector.tensor_copy(w1_sb[:], w1_st[:])
    b1_sb = sb.tile([HID, 1], f32, tag="b1")
    nc.sync.dma_start(b1_sb[:], b1_d)
    w2_sb = sb.tile([HID, 1], f32, tag="w2")
    nc.sync.dma_start(w2_sb[:], w2_d)
    b2_sb = sb.tile([1, 1], f32, tag="b2")
    nc.sync.dma_start(b2_sb[:], b2_d)

    # broadcast b2 across partitions via PE: ones[1,128].T @ b2[1,1]
    ones_sb = sb.tile([1, 128], f32, tag="ones")
    nc.vector.memset(ones_sb[:], 1.0)
    pbc = ps.tile([128, 1], f32, tag="tp", bufs=4, name="pbc")
    nc.tensor.matmul(pbc[:], ones_sb[:], b2_sb[:], start=True, stop=True)
    b2b_sb = sb.tile([128, 1], f32, tag="b2b")
    nc.vector.tensor_copy(b2b_sb[:], pbc[:])

    # ---------------- h -> h_m (f32r stationary) ----------------
    h_m = sb.tile([128, JT, D], f32r, tag="h_m")        # 64 KB/part
    h_v = h_d.rearrange("(r p) d -> p r d", p=128)
    for t in range(16):
        hst = sb.tile([128, 4, D], f32, tag="hst", bufs=4, name=f"hst_{t}")
        nc.sync.dma_start(hst[:], h_v[:, 4 * t:4 * (t + 1), :])
        nc.vector.tensor_copy(h_m[:, 4 * t:4 * (t + 1), :], hst[:])

    # ------------- sharded MLP: e for this core's 1024 nodes -------------
    # h_mlp [1024, 256] = this core's h rows (different per core).
    hl_st = sb.tile([128, LJT, D], f32, tag="hlst")     # 8 KB/part
    nc.sync.dma_start(hl_st[:], hm_d.rearrange("(r p) d -> p r d", p=128))
    hl_m = sb.tile([128, LJT, D], f32r, tag="hlm")
    nc.vector.tensor_copy(hl_m[:], hl_st[:])

    zT_sb = sb.tile([HID, R], f32, tag="zT")            # 4 KB/part
    for jc in range(2):                                 # local j-chunks of 512
        pz = ps.tile([HID, 512], f32, tag="tp", bufs=4, name=f"pz_{jc}")
        for b in range(DB):
            pt = ps.tile([128, 512], f32, tag="tp", bufs=4, name=f"tph_{jc}_{b}")
            for s in range(4):
                # f32r regular-mm transpose (4 cyc/row at N=128, stays warm)
                nc.tensor.matmul(pt[:, 128 * s:128 * (s + 1)],
                                 hl_m[:, 4 * jc + s, 128 * b:128 * (b + 1)],
                                 ident_r[:], start=True, stop=True)
            hTb = sb.tile([128, 512], f32r, tag="hTb", bufs=2, name=f"hTb_{jc}_{b}")
            nc.vector.tensor_copy(hTb[:], pt[:])
            nc.tensor.matmul(pz[:], w1_sb[:, b, :], hTb[:],
                             start=(b == 0), stop=(b == DB - 1))
        nc.vector.tensor_scalar(zT_sb[:, 512 * jc:512 * (jc + 1)], pz[:],
                                b1_sb[:], 0.0, ALU.add, ALU.max)

    pu = ps.tile([128, LJT], f32, tag="tp", bufs=4, name="pu")
    for r in range(LJT):
        nc.tensor.matmul(pu[:, r:r + 1], zT_sb[:, 128 * r:128 * (r + 1)], w2_sb[:],
                         start=True, stop=True)
    e_loc = sb.tile([128, LJT], f32, tag="e_loc")
    nc.scalar.activation(e_loc[:], pu[:], AF.Sigmoid, bias=b2b_sb[:], scale=1.0)

    # AllGather e across the 8 cores (DRAM bounce, [1024] -> [8192])
    e_in_b = dram.tile([R], f32, name="e_in_b")
    e_out_b = dram.tile([N], f32, name="e_out_b")
    nc.sync.dma_start(e_in_b[:].rearrange("(r p) -> p (r)", p=128), e_loc[:])
    nc.gpsimd.collective_compute(
        "AllGather", ALU.bypass, replica_groups=[list(range(NCORES))],
        ins=[e_in_b.opt()], outs=[e_out_b.opt()])
    # full e to SBUF column layout + to the e output
    e_sb = sb.tile([128, JT], f32, tag="e")
    nc.sync.dma_start(e_sb[:], e_out_b[:].rearrange("(r p) -> p (r)", p=128))
    nc.sync.dma_start(e_d[:, 0], e_out_b[:])

    # ------- phase 1: bf16 mask, regular-mm transpose, e-scaled evac, matmul -------
    for ic in range(NIC):
        accs = [ps.tile([128, 512], f32, tag=f"acc{b}", bufs=2, name=f"acc{b}_{ic}")
                for b in range(DB)]
        for jt2 in range(NJC2):
            masks_ = []
            for k in range(4):
                at = sb.tile([128, 1024], f32, tag="a_in", bufs=6,
                             name=f"a_{ic}_{jt2}_{k}")
                nc.sync.dma_start(
                    at[:],
                    a_d[512 * ic + 128 * k: 512 * ic + 128 * (k + 1),
                        1024 * jt2: 1024 * (jt2 + 1)])
                mt = sb.tile([128, 1024], bf16, tag="mask", bufs=6,
                             name=f"m_{ic}_{jt2}_{k}")
                if (jt2 * 4 + k) % 8 < 3:
                    nc.vector.tensor_scalar(mt[:], at[:], 0.0, None, ALU.is_gt)
                else:
                    nc.scalar.sign(mt[:], at[:])
                masks_.append(mt)
            for q in range(8):
                j = 8 * jt2 + q
                pt = ps.tile([128, 512], f32, tag="tp", bufs=4,
                             name=f"tp_{ic}_{j}")
                for k in range(4):
                    # regular matmul transpose: out = mask.T @ I (keeps HAM warm)
                    nc.tensor.matmul(pt[:, 128 * k:128 * (k + 1)],
                                     masks_[k][:, 128 * q:128 * (q + 1)],
                                     ident_bf[:], start=True, stop=True)
                aT = sb.tile([128, 512], f32r, tag="attT", bufs=8,
                             name=f"attT_{ic}_{j}")
                # evacuate with per-partition scale by e[j] (mask is exact 0/1)
                if q % 2 == 1:
                    nc.scalar.mul(aT[:], pt[:], e_sb[:, j:j + 1])
                else:
                    nc.vector.tensor_scalar_mul(aT[:], pt[:], e_sb[:, j:j + 1])
                for b in range(DB):
                    nc.tensor.matmul(accs[b][:],
                                     h_m[:, j, 128 * b:128 * (b + 1)], aT[:],
                                     start=(j == 0), stop=(j == JT - 1))
        for b in range(DB):
            ot = sb.tile([128, 512], f32, tag="hout", bufs=4,
                         name=f"ot_{ic}_{b}")
            nc.vector.tensor_copy(ot[:], accs[b][:])
            nc.sync.dma_start(
                houtT_d[128 * b:128 * (b + 1), 512 * ic:512 * (ic + 1)], ot[:])

    ctx.close()


_built = None


def _build():
    global _built
    if _built is not None:
        return _built
    nc = bacc.Bacc(get_trn_type(), target_bir_lowering=False, debug=False,
                   num_devices=NCORES)
    a_d = nc.dram_tensor("a_shard", [R, N], f32, kind="ExternalInput").ap()
    h_d = nc.dram_tensor("h_full", [N, D], f32, kind="ExternalInput").ap()
    hm_d = nc.dram_tensor("h_mlp", [R, D], f32, kind="ExternalInput").ap()
    w1_d = nc.dram_tensor("w1", [D, HID], f32, kind="ExternalInput").ap()
    b1_d = nc.dram_tensor("b1", [HID, 1], f32, kind="ExternalInput").ap()
    w2_d = nc.dram_tensor("w2", [HID, 1], f32, kind="ExternalInput").ap()
    b2_d = nc.dram_tensor("b2", [1, 1], f32, kind="ExternalInput").ap()
    houtT_d = nc.dram_tensor("houtT", [D, R], f32, kind="ExternalOutput").ap()
    e_d = nc.dram_tensor("e_out", [N, 1], f32, kind="ExternalOutput").ap()

    with tile.TileContext(nc) as tc:
        _emit(tc, a_d, h_d, hm_d, w1_d, b1_d, w2_d, b2_d, houtT_d, e_d)
    nc.compile()
    _built = nc
    return nc


def _run(nc, graph_info, h, W1, b1, W2, b2, **run_kwargs):
    in_maps = []
    for c in range(NCORES):
        in_maps.append({
            "a_shard": np.ascontiguousarray(graph_info[c * R:(c + 1) * R]),
            "h_full": h,
            "h_mlp": np.ascontiguousarray(h[c * R:(c + 1) * R]),
            "w1": W1,
            "b1": b1.reshape(HID, 1),
            "w2": W2.reshape(HID, 1),
            "b2": b2.reshape(1, 1),
        })
    res = run_bass_kernel_spmd(nc, in_maps, list(range(NCORES)), **run_kwargs)
    return res


def kernel(graph_info, h, W1, b1, W2, b2):
    graph_info = np.asarray(graph_info, dtype=np.float32)
    h = np.asarray(h, dtype=np.float32)
    W1 = np.asarray(W1, dtype=np.float32)
    b1 = np.asarray(b1, dtype=np.float32)
    W2 = np.asarray(W2, dtype=np.float32)
    b2 = np.asarray(b2, dtype=np.float32)

    nc = _build()
    res = _run(nc, graph_info, h, W1, b1, W2, b2)
    h_out = np.concatenate([r["houtT"].T for r in res.results], axis=0)
    e = res.results[0]["e_out"]
    return (h_out, e)


# revision 15
# speedup vs baseline: 2.4372x; 1.0001x over previous
"""TRN2 Bass kernel for nn_Att_mlp_sigmod (gnn message passing).

Reference computation:
    e = sigmoid(relu(h @ W1 + b1) @ W2 + b2)          # [N, 1]
    att[i, j] = e[j] * (graph_info[i, j] > 0)
    h_out = att @ h                                   # [N, D]
    returns (h_out, e)

Strategy (8 NeuronCores, SPMD, no collectives):
  - Row-shard graph_info (core c owns rows [c*1024, (c+1)*1024)); replicate
    h + weights; every core computes the full e locally (a cross-core
    AllGather was measured at ~100us+ latency under this runtime — far more
    than the ~35us of redundant per-core MLP it would save).
  - Core output: houtT[d, i] = sum_j g[j, d] * maskT[j, i] with
    g = e * (h rounded to float32r) scaled IN PLACE (so the mask pipeline
    never waits on e) and maskT = (A^T > 0) in exact {0,1} f32r.
  - mask pipeline: A tiles -> exact {0,1} bf16 masks (big DVE is_gt / ACT
    Sign ops) -> transposed on the PE as REGULAR matmuls against a bf16
    identity (1 cyc/row + FWL; regular matmuls keep the HAM clock at 2.4
    GHz — is_transpose-mode ops count as idle and halve the clock) -> each
    [128,512] f32 PSUM bank (exact 0/1) evacuated by ONE plain copy into
    f32r attT (DVE/ACT split). Transposes and matmuls are emitted in long
    batches per 1024-wide j-chunk for FWL and reorder-window pipelining.
  - f32r gives ~1e-4 relative error at full bf16 matmul speed (1 cyc/row
    for moving free >= 256).
  - Final unshard in numpy: transpose + concat of per-core houtT.
"""

import hashlib
import os
import shutil
import numpy as np
from contextlib import ExitStack

import concourse.bass as bass
import concourse.tile as tile
from concourse import bacc, mybir
from concourse.bass_utils import run_bass_kernel_spmd
from concourse.masks import make_identity
from concourse._compat import get_trn_type

# Cache compiled NEFFs by BIR hash — walrus compile of this kernel takes
# minutes and bass2jax recompiles in a fresh tempdir on every process start.
import concourse.bass2jax as _b2j

_NEFF_CACHE = "/tmp/bass_neff_cache"

if not getattr(_b2j, "_ant_neff_cache_installed", False):
    _orig_cbk = _b2j.compile_bir_kernel

    def _cached_cbk(bir_json, tmpdir, neff_name="file.neff"):
        os.makedirs(_NEFF_CACHE, exist_ok=True)
        key = hashlib.sha256(bir_json).hexdigest()[:24]
        cpath = os.path.join(_NEFF_CACHE, f"{key}.neff")
        dst = os.path.join(tmpdir, neff_name)
        if os.path.exists(cpath):
            shutil.copy(cpath, dst)
            return dst
        neff = _orig_cbk(bir_json, tmpdir, neff_name)
        tmp = f"{cpath}.tmp{os.getpid()}"
        shutil.copy(neff, tmp)
        os.replace(tmp, cpath)
        return neff

    _b2j.compile_bir_kernel = _cached_cbk
    _b2j._ant_neff_cache_installed = True

N, D, HID = 8192, 256, 64
NCORES = 8
R = N // NCORES          # 1024 rows of A per core
JT = N // 128            # 64 j-tiles
NJC2 = N // 1024         # 8 j-chunks of 1024
NIC = R // 512           # 2 i-chunks per core
DB = D // 128            # 2 d-blocks

f32 = mybir.dt.float32
f32r = mybir.dt.float32r
bf16 = mybir.dt.bfloat16

AF = mybir.ActivationFunctionType
ALU = mybir.AluOpType


def _emit(tc, a_d, h_d, w1_d, b1_d, w2_d, b2_d, houtT_d, e_d):
    nc = tc.nc
    ctx = ExitStack()

    sb = ctx.enter_context(tc.tile_pool(name="sb", bufs=1))
    ps = ctx.enter_context(tc.tile_pool(name="ps", bufs=1, space="PSUM"))

    # ---------------- constants ----------------
    ident_f = sb.tile([128, 128], f32, tag="ident_f")
    make_identity(nc, ident_f[:])
    ident_r = sb.tile([128, 128], f32r, tag="ident_r")
    nc.vector.tensor_copy(ident_r[:], ident_f[:])
    ident_bf = sb.tile([128, 128], bf16, tag="ident_bf")
    nc.vector.tensor_copy(ident_bf[:], ident_f[:])

    w1_st = sb.tile([128, DB, HID], f32, tag="w1st")
    nc.sync.dma_start(w1_st[:], w1_d.rearrange("(b p) h -> p b h", p=128))
    w1_sb = sb.tile([128, DB, HID], f32r, tag="w1")
    nc.vector.tensor_copy(w1_sb[:], w1_st[:])
    b1_sb = sb.tile([HID, 1], f32, tag="b1")
    nc.sync.dma_start(b1_sb[:], b1_d)
    w2_sb = sb.tile([HID, 1], f32, tag="w2")
    nc.sync.dma_start(w2_sb[:], w2_d)
    b2_sb = sb.tile([1, 1], f32, tag="b2")
    nc.sync.dma_start(b2_sb[:], b2_d)

    # broadcast b2 across partitions via PE: ones[1,128].T @ b2[1,1]
    ones_sb = sb.tile([1, 128], f32, tag="ones")
    nc.vector.memset(ones_sb[:], 1.0)
    pbc = ps.tile([128, 1], f32, tag="tp", bufs=4, name="pbc")
    nc.tensor.matmul(pbc[:], ones_sb[:], b2_sb[:], start=True, stop=True)
    b2b_sb = sb.tile([128, 1], f32, tag="b2b")
    nc.vector.tensor_copy(b2b_sb[:], pbc[:])

    # ---------------- h -> h_m (f32r stationary) ----------------
    h_m = sb.tile([128, JT, D], f32r, tag="h_m")        # 64 KB/part
    h_v = h_d.rearrange("(r p) d -> p r d", p=128)
    for t in range(16):
        hst = sb.tile([128, 4, D], f32, tag="hst", bufs=4, name=f"hst_{t}")
        nc.sync.dma_start(hst[:], h_v[:, 4 * t:4 * (t + 1), :])
        nc.vector.tensor_copy(h_m[:, 4 * t:4 * (t + 1), :], hst[:])

    # ------------- replicated MLP: full e on every core -------------
    zT_sb = sb.tile([HID, N], f32, tag="zT")            # 32 KB/part
    for jc in range(16):                                # j-chunks of 512
        pz = ps.tile([HID, 512], f32, tag="tp", bufs=4, name=f"pz_{jc}")
        for b in range(DB):
            pt = ps.tile([128, 512], f32, tag="tp", bufs=4, name=f"tph_{jc}_{b}")
            for s in range(4):
                # f32r regular-mm transpose (keeps HAM warm)
                nc.tensor.matmul(pt[:, 128 * s:128 * (s + 1)],
                                 h_m[:, 4 * jc + s, 128 * b:128 * (b + 1)],
                                 ident_r[:], start=True, stop=True)
            hTb = sb.tile([128, 512], f32r, tag="hTb", bufs=2, name=f"hTb_{jc}_{b}")
            nc.vector.tensor_copy(hTb[:], pt[:])
            nc.tensor.matmul(pz[:], w1_sb[:, b, :], hTb[:],
                             start=(b == 0), stop=(b == DB - 1))
        nc.vector.tensor_scalar(zT_sb[:, 512 * jc:512 * (jc + 1)], pz[:],
                                b1_sb[:], 0.0, ALU.add, ALU.max)

    pu = ps.tile([128, JT], f32, tag="tp", bufs=4, name="pu")
    for r in range(JT):
        nc.tensor.matmul(pu[:, r:r + 1], zT_sb[:, 128 * r:128 * (r + 1)], w2_sb[:],
                         start=True, stop=True)
    e_sb = sb.tile([128, JT], f32, tag="e")
    nc.scalar.activation(e_sb[:], pu[:], AF.Sigmoid, bias=b2b_sb[:], scale=1.0)
    # e output: e_dram[r*128 + p] = e_sb[p, r]
    nc.sync.dma_start(e_d.rearrange("(r p) o -> p (r o)", p=128), e_sb[:])

    # g = e * h_m in place (per-partition scale; only the main matmuls wait)
    for t in range(4):
        nc.vector.tensor_tensor(
            h_m[:, 16 * t:16 * (t + 1), :],
            h_m[:, 16 * t:16 * (t + 1), :],
            e_sb[:, 16 * t:16 * (t + 1), None].to_broadcast([128, 16, D]),
            ALU.mult)

    # ------- phase 1: bf16 mask -> batched transposes -> copy evac -> matmul -------
    for ic in range(NIC):
        accs = [ps.tile([128, 512], f32, tag=f"acc{b}", bufs=2, name=f"acc{b}_{ic}")
                for b in range(DB)]
        for jt2 in range(NJC2):
            masks_ = []
            for k in range(4):
                at = sb.tile([128, 1024], f32, tag="a_in", bufs=6,
                             name=f"a_{ic}_{jt2}_{k}")
                nc.sync.dma_start(
                    at[:],
                    a_d[512 * ic + 128 * k: 512 * ic + 128 * (k + 1),
                        1024 * jt2: 1024 * (jt2 + 1)])
                mt = sb.tile([128, 1024], bf16, tag="mask", bufs=6,
                             name=f"m_{ic}_{jt2}_{k}")
                if (jt2 * 4 + k) % 8 < 3:
                    nc.vector.tensor_scalar(mt[:], at[:], 0.0, None, ALU.is_gt)
                else:
                    nc.scalar.sign(mt[:], at[:])
                masks_.append(mt)
            # two halves of 4 j-blocks each: 16 transposes + 4 evacs, then 8 matmuls
            for half in range(2):
                aTs = []
                for qq in range(4):
                    q = 4 * half + qq
                    j = 8 * jt2 + q
                    pt = ps.tile([128, 512], f32, tag="tp", bufs=4,
                                 name=f"tp_{ic}_{j}")
                    for k in range(4):
                        # regular matmul transpose (out = mask.T @ I)
                        nc.tensor.matmul(pt[:, 128 * k:128 * (k + 1)],
                                         masks_[k][:, 128 * q:128 * (q + 1)],
                                         ident_bf[:], start=True, stop=True)
                    aT = sb.tile([128, 512], f32r, tag="attT", bufs=16,
                                 name=f"attT_{ic}_{j}")
                    # plain evacuation: PSUM holds exact 0/1, round to f32r
                    if q % 2 == 1:
                        nc.scalar.copy(aT[:], pt[:])
                    else:
                        nc.vector.tensor_copy(aT[:], pt[:])
                    aTs.append((j, aT))
                for j, aT in aTs:
                    for b in range(DB):
                        nc.tensor.matmul(accs[b][:],
                                         h_m[:, j, 128 * b:128 * (b + 1)], aT[:],
                                         start=(j == 0), stop=(j == JT - 1))
        for b in range(DB):
            ot = sb.tile([128, 512], f32, tag="hout", bufs=4,
                         name=f"ot_{ic}_{b}")
            nc.vector.tensor_copy(ot[:], accs[b][:])
            nc.sync.dma_start(
                houtT_d[128 * b:128 * (b + 1), 512 * ic:512 * (ic + 1)], ot[:])

    ctx.close()


_built = None


def _build():
    global _built
    if _built is not None:
        return _built
    nc = bacc.Bacc(get_trn_type(), target_bir_lowering=False, debug=False)
    a_d = nc.dram_tensor("a_shard", [R, N], f32, kind="ExternalInput").ap()
    h_d = nc.dram_tensor("h_full", [N, D], f32, kind="ExternalInput").ap()
    w1_d = nc.dram_tensor("w1", [D, HID], f32, kind="ExternalInput").ap()
    b1_d = nc.dram_tensor("b1", [HID, 1], f32, kind="ExternalInput").ap()
    w2_d = nc.dram_tensor("w2", [HID, 1], f32, kind="ExternalInput").ap()
    b2_d = nc.dram_tensor("b2", [1, 1], f32, kind="ExternalInput").ap()
    houtT_d = nc.dram_tensor("houtT", [D, R], f32, kind="ExternalOutput").ap()
    e_d = nc.dram_tensor("e_out", [N, 1], f32, kind="ExternalOutput").ap()

    with tile.TileContext(nc) as tc:
        _emit(tc, a_d, h_d, w1_d, b1_d, w2_d, b2_d, houtT_d, e_d)
    nc.compile()
    _built = nc
    return nc


def _run(nc, graph_info, h, W1, b1, W2, b2, **run_kwargs):
    in_maps = []
    for c in range(NCORES):
        in_maps.append({
            "a_shard": np.ascontiguousarray(graph_info[c * R:(c + 1) * R]),
            "h_full": h,
            "w1": W1,
            "b1": b1.reshape(HID, 1),
            "w2": W2.reshape(HID, 1),
            "b2": b2.reshape(1, 1),
        })
    res = run_bass_kernel_spmd(nc, in_maps, list(range(NCORES)), **run_kwargs)
    return res


def kernel(graph_info, h, W1, b1, W2, b2):
    graph_info = np.asarray(graph_info, dtype=np.float32)
    h = np.asarray(h, dtype=np.float32)
    W1 = np.asarray(W1, dtype=np.float32)
    b1 = np.asarray(b1, dtype=np.float32)
    W2 = np.asarray(W2, dtype=np.float32)
    b2 = np.asarray(b2, dtype=np.float32)

    nc = _build()
    res = _run(nc, graph_info, h, W1, b1, W2, b2)
    h_out = np.concatenate([r["houtT"].T for r in res.results], axis=0)
    e = res.results[0]["e_out"]
    return (h_out, e)


# revision 21
# speedup vs baseline: 2.4430x; 1.0024x over previous
"""TRN2 Bass kernel for nn_Att_mlp_sigmod (gnn message passing).

Reference computation:
    e = sigmoid(relu(h @ W1 + b1) @ W2 + b2)          # [N, 1]
    att[i, j] = e[j] * (graph_info[i, j] > 0)
    h_out = att @ h                                   # [N, D]
    returns (h_out, e)

Strategy (8 NeuronCores, SPMD, no collectives):
  - Row-shard graph_info (core c owns rows [c*1024, (c+1)*1024)); replicate
    h + weights; every core computes the full e locally (a cross-core
    AllGather was measured at ~100us+ latency under this runtime — far more
    than the ~35us of redundant per-core MLP it would save).
  - Core output: houtT[d, i] = sum_j g[j, d] * maskT[j, i] with
    g = e * (h rounded to float32r) scaled IN PLACE (so the mask pipeline
    never waits on e) and maskT = (A^T > 0) in exact {0,1} f32r.
  - mask pipeline: A tiles -> exact {0,1} bf16 masks (big DVE is_gt / ACT
    Sign ops) -> transposed on the PE as REGULAR matmuls against a bf16
    identity (1 cyc/row + FWL; regular matmuls keep the HAM clock at 2.4
    GHz — is_transpose-mode ops count as idle and halve the clock) -> each
    [128,512] f32 PSUM bank (exact 0/1) evacuated by ONE plain copy into
    f32r attT (DVE/ACT split). Transposes and matmuls are emitted in long
    batches per 1024-wide j-chunk for FWL and reorder-window pipelining.
  - f32r gives ~1e-4 relative error at full bf16 matmul speed (1 cyc/row
    for moving free >= 256).
  - Final unshard in numpy: transpose + concat of per-core houtT.
"""

import hashlib
import os
import shutil
import numpy as np
from contextlib import ExitStack

import concourse.bass as bass
import concourse.tile as tile
from concourse import bacc, mybir
from concourse.bass_utils import run_bass_kernel_spmd
from concourse.masks import make_identity
from concourse._compat import get_trn_type

# Cache compiled NEFFs by BIR hash — walrus compile of this kernel takes
# minutes and bass2jax recompiles in a fresh tempdir on every process start.
import concourse.bass2jax as _b2j

_NEFF_CACHE = "/tmp/bass_neff_cache"

if not getattr(_b2j, "_ant_neff_cache_installed", False):
    _orig_cbk = _b2j.compile_bir_kernel

    def _cached_cbk(bir_json, tmpdir, neff_name="file.neff"):
        os.makedirs(_NEFF_CACHE, exist_ok=True)
        key = hashlib.sha256(bir_json).hexdigest()[:24]
        cpath = os.path.join(_NEFF_CACHE, f"{key}.neff")
        dst = os.path.join(tmpdir, neff_name)
        if os.path.exists(cpath):
            shutil.copy(cpath, dst)
            return dst
        neff = _orig_cbk(bir_json, tmpdir, neff_name)
        tmp = f"{cpath}.tmp{os.getpid()}"
        shutil.copy(neff, tmp)
        os.replace(tmp, cpath)
        return neff

    _b2j.compile_bir_kernel = _cached_cbk
    _b2j._ant_neff_cache_installed = True

N, D, HID = 8192, 256, 64
NCORES = 8
R = N // NCORES          # 1024 rows of A per core
JT = N // 128            # 64 j-tiles
NJC2 = N // 1024         # 8 j-chunks of 1024
NIC = R // 512           # 2 i-chunks per core
DB = D // 128            # 2 d-blocks

f32 = mybir.dt.float32
f32r = mybir.dt.float32r
bf16 = mybir.dt.bfloat16

AF = mybir.ActivationFunctionType
ALU = mybir.AluOpType


def _emit(tc, a_d, h_d, w1_d, b1_d, w2_d, b2_d, houtT_d, e_d):
    nc = tc.nc
    ctx = ExitStack()

    sb = ctx.enter_context(tc.tile_pool(name="sb", bufs=1))
    ps = ctx.enter_context(tc.tile_pool(name="ps", bufs=1, space="PSUM"))

    # ---------------- constants ----------------
    ident_f = sb.tile([128, 128], f32, tag="ident_f")
    make_identity(nc, ident_f[:])
    ident_r = sb.tile([128, 128], f32r, tag="ident_r")
    nc.vector.tensor_copy(ident_r[:], ident_f[:])
    ident_bf = sb.tile([128, 128], bf16, tag="ident_bf")
    nc.vector.tensor_copy(ident_bf[:], ident_f[:])

    w1_st = sb.tile([128, DB, HID], f32, tag="w1st")
    nc.sync.dma_start(w1_st[:], w1_d.rearrange("(b p) h -> p b h", p=128))
    w1_sb = sb.tile([128, DB, HID], f32r, tag="w1")
    nc.vector.tensor_copy(w1_sb[:], w1_st[:])
    b1_sb = sb.tile([HID, 1], f32, tag="b1")
    nc.sync.dma_start(b1_sb[:], b1_d)
    w2_sb = sb.tile([HID, 1], f32, tag="w2")
    nc.sync.dma_start(w2_sb[:], w2_d)
    b2_sb = sb.tile([1, 1], f32, tag="b2")
    nc.sync.dma_start(b2_sb[:], b2_d)

    # broadcast b2 across partitions via PE: ones[1,128].T @ b2[1,1]
    ones_sb = sb.tile([1, 128], f32, tag="ones")
    nc.vector.memset(ones_sb[:], 1.0)
    pbc = ps.tile([128, 1], f32, tag="tp", bufs=4, name="pbc")
    nc.tensor.matmul(pbc[:], ones_sb[:], b2_sb[:], start=True, stop=True)
    b2b_sb = sb.tile([128, 1], f32, tag="b2b")
    nc.vector.tensor_copy(b2b_sb[:], pbc[:])

    # ---------------- h -> h_m (f32r stationary) ----------------
    h_m = sb.tile([128, JT, D], f32r, tag="h_m")        # 64 KB/part
    h_v = h_d.rearrange("(r p) d -> p r d", p=128)
    for t in range(16):
        hst = sb.tile([128, 4, D], f32, tag="hst", bufs=4, name=f"hst_{t}")
        nc.sync.dma_start(hst[:], h_v[:, 4 * t:4 * (t + 1), :])
        nc.vector.tensor_copy(h_m[:, 4 * t:4 * (t + 1), :], hst[:])

    # ------------- replicated MLP: full e on every core -------------
    zT_sb = sb.tile([HID, N], f32, tag="zT")            # 32 KB/part
    for jc in range(16):                                # j-chunks of 512
        pz = ps.tile([HID, 512], f32, tag="tp", bufs=4, name=f"pz_{jc}")
        for b in range(DB):
            pt = ps.tile([128, 512], f32, tag="tp", bufs=4, name=f"tph_{jc}_{b}")
            for s in range(4):
                # f32r regular-mm transpose (keeps HAM warm)
                nc.tensor.matmul(pt[:, 128 * s:128 * (s + 1)],
                                 h_m[:, 4 * jc + s, 128 * b:128 * (b + 1)],
                                 ident_r[:], start=True, stop=True)
            hTb = sb.tile([128, 512], f32r, tag="hTb", bufs=2, name=f"hTb_{jc}_{b}")
            nc.vector.tensor_copy(hTb[:], pt[:])
            nc.tensor.matmul(pz[:], w1_sb[:, b, :], hTb[:],
                             start=(b == 0), stop=(b == DB - 1))
        nc.vector.tensor_scalar(zT_sb[:, 512 * jc:512 * (jc + 1)], pz[:],
                                b1_sb[:], 0.0, ALU.add, ALU.max)

    pu = ps.tile([128, JT], f32, tag="tp", bufs=4, name="pu")
    for r in range(JT):
        nc.tensor.matmul(pu[:, r:r + 1], zT_sb[:, 128 * r:128 * (r + 1)], w2_sb[:],
                         start=True, stop=True)
    e_sb = sb.tile([128, JT], f32, tag="e")
    nc.scalar.activation(e_sb[:], pu[:], AF.Sigmoid, bias=b2b_sb[:], scale=1.0)
    # e output: e_dram[r*128 + p] = e_sb[p, r]
    nc.sync.dma_start(e_d.rearrange("(r p) o -> p (r o)", p=128), e_sb[:])

    # g = e * h_m in place (per-partition scale; only the main matmuls wait)
    for t in range(4):
        nc.vector.tensor_tensor(
            h_m[:, 16 * t:16 * (t + 1), :],
            h_m[:, 16 * t:16 * (t + 1), :],
            e_sb[:, 16 * t:16 * (t + 1), None].to_broadcast([128, 16, D]),
            ALU.mult)

    # ------- phase 1: bf16 mask -> batched transposes -> copy evac -> matmul -------
    for ic in range(NIC):
        accs = [ps.tile([128, 512], f32, tag=f"acc{b}", bufs=2, name=f"acc{b}_{ic}")
                for b in range(DB)]
        for jt2 in range(NJC2):
            masks_ = []
            for k in range(4):
                at = sb.tile([128, 1024], f32, tag="a_in", bufs=6,
                             name=f"a_{ic}_{jt2}_{k}")
                nc.sync.dma_start(
                    at[:],
                    a_d[512 * ic + 128 * k: 512 * ic + 128 * (k + 1),
                        1024 * jt2: 1024 * (jt2 + 1)])
                mt = sb.tile([128, 1024], bf16, tag="mask", bufs=6,
                             name=f"m_{ic}_{jt2}_{k}")
                if (jt2 * 4 + k) % 8 < 3:
                    nc.vector.tensor_scalar(mt[:], at[:], 0.0, None, ALU.is_gt)
                else:
                    nc.scalar.sign(mt[:], at[:])
                masks_.append(mt)
            # two halves of 4 j-blocks each: 16 transposes + 4 evacs, then 8 matmuls
            for half in range(2):
                aTs = []
                for qq in range(4):
                    q = 4 * half + qq
                    j = 8 * jt2 + q
                    pt = ps.tile([128, 512], f32, tag="tp", bufs=4,
                                 name=f"tp_{ic}_{j}")
                    for k in range(4):
                        # regular matmul transpose (out = mask.T @ I)
                        nc.tensor.matmul(pt[:, 128 * k:128 * (k + 1)],
                                         masks_[k][:, 128 * q:128 * (q + 1)],
                                         ident_bf[:], start=True, stop=True)
                    aT = sb.tile([128, 512], f32r, tag="attT", bufs=16,
                                 name=f"attT_{ic}_{j}")
                    # plain evacuation: PSUM holds exact 0/1, round to f32r
                    if q % 2 == 1:
                        nc.scalar.copy(aT[:], pt[:])
                    else:
                        nc.vector.tensor_copy(aT[:], pt[:])
                    aTs.append((j, aT))
                for j, aT in aTs:
                    for b in range(DB):
                        nc.tensor.matmul(accs[b][:],
                                         h_m[:, j, 128 * b:128 * (b + 1)], aT[:],
                                         start=(j == 0), stop=(j == JT - 1))
        for b in range(DB):
            ot = sb.tile([128, 512], f32, tag="hout", bufs=4,
                         name=f"ot_{ic}_{b}")
            nc.vector.tensor_copy(ot[:], accs[b][:])
            nc.sync.dma_start(
                houtT_d[128 * b:128 * (b + 1), 512 * ic:512 * (ic + 1)], ot[:])

    ctx.close()


_built = None


def _build():
    global _built
    if _built is not None:
        return _built
    nc = bacc.Bacc(get_trn_type(), target_bir_lowering=False, debug=False)
    a_d = nc.dram_tensor("a_shard", [R, N], f32, kind="ExternalInput").ap()
    h_d = nc.dram_tensor("h_full", [N, D], f32, kind="ExternalInput").ap()
    w1_d = nc.dram_tensor("w1", [D, HID], f32, kind="ExternalInput").ap()
    b1_d = nc.dram_tensor("b1", [HID, 1], f32, kind="ExternalInput").ap()
    w2_d = nc.dram_tensor("w2", [HID, 1], f32, kind="ExternalInput").ap()
    b2_d = nc.dram_tensor("b2", [1, 1], f32, kind="ExternalInput").ap()
    houtT_d = nc.dram_tensor("houtT", [D, R], f32, kind="ExternalOutput").ap()
    e_d = nc.dram_tensor("e_out", [N, 1], f32, kind="ExternalOutput").ap()

    with tile.TileContext(nc) as tc:
        _emit(tc, a_d, h_d, w1_d, b1_d, w2_d, b2_d, houtT_d, e_d)
    nc.compile()
    _built = nc
    return nc


def _run(nc, graph_info, h, W1, b1, W2, b2, **run_kwargs):
    in_maps = []
    for c in range(NCORES):
        in_maps.append({
            "a_shard": np.ascontiguousarray(graph_info[c * R:(c + 1) * R]),
            "h_full": h,
            "w1": W1,
            "b1": b1.reshape(HID, 1),
            "w2": W2.reshape(HID, 1),
            "b2": b2.reshape(1, 1),
        })
    res = run_bass_kernel_spmd(nc, in_maps, list(range(NCORES)), **run_kwargs)
    return res


def kernel(graph_info, h, W1, b1, W2, b2):
    graph_info = np.asarray(graph_info, dtype=np.float32)
    h = np.asarray(h, dtype=np.float32)
    W1 = np.asarray(W1, dtype=np.float32)
    b1 = np.asarray(b1, dtype=np.float32)
    W2 = np.asarray(W2, dtype=np.float32)
    b2 = np.asarray(b2, dtype=np.float32)

    nc = _build()
    res = _run(nc, graph_info, h, W1, b1, W2, b2)
    h_out = np.concatenate([r["houtT"].T for r in res.results], axis=0)
    e = res.results[0]["e_out"]
    return (h_out, e)
